# revision 7
# baseline (speedup 1.0000x reference)
"""Trainium2 Bass kernel for nn_Neuron_33500744909351 (scatter_memory).

Reference computation (per full batch B=8192, I=4096, C=128, CMS=4, R=16):
  dist   = context_maps @ context_inputs                    # (4, B)
  bits   = dist > context_bias                              # (4, B)
  idx    = sum(bits * 2^k)                                  # (B,) in [0,16)
  out[b] = weights[idx[b]] . logits[:, b], clipped to +-logit(0.01)
  g[b]   = LR * (sigmoid(out[b]) - targets[b])
  new_w  = clip(weights - scatter_add(g[b] * logits[:, b] -> row idx[b]), -5, 5)
  (sequential-scan clip never binds mid-scan at these scales, so the
   scatter-add + single final clip is exact w.h.p.)

Sharding: data-parallel over batch. Each of the 8 cores takes B_SH=1024
batch columns, computes its out-shard plus a partial delta (16, 4096);
host sums partials and applies the final clip.

Device-side layout is batch-major ("T" = batch on partitions). The host
supplies logitsT (pre-transposed) so both matmuls stream natural SBUF
layouts:
  mm1: allout (16, B_SH)  = weightsT.T @ logits    (contract input dim)
  mm2: delta  (16, 4096) += coefT.T @ logitsT      (contract batch dim)
"""

import numpy as np

import concourse.bass as bass
import concourse.bacc as bacc
import concourse.mybir as mybir
import concourse.tile as tile
from concourse.bass_utils import run_bass_kernel_spmd

F32 = mybir.dt.float32

N_CORES = 8
B = 8192
B_SH = B // N_CORES          # 1024 batch per core
I = 4096                     # input_size
C = 128                      # context_size
CMS = 4
R = 16                       # weight rows
P = 128                      # partitions
NB = B_SH // P               # 8 batch chunks of 128
NT = I // P                  # 32 input-dim chunks of 128

LR = 0.01
WCLIP = 5.0


def _logit_clip_f32():
    # match reference: lo = log(0.01) - log1p(-0.01) in float32 arithmetic
    p = np.float32(0.01)
    lo = np.float32(np.log(p, dtype=np.float32) - np.log1p(-p, dtype=np.float32))
    return float(lo), float(-lo)


CLIP_LO, CLIP_HI = _logit_clip_f32()


def build(nc: bass.Bass):
    """Emit the per-core kernel. Returns dict of dram tensor names."""
    # ---- per-core DRAM inputs ----
    lg = nc.dram_tensor("lg", [I, B_SH], F32, kind="ExternalInput").ap()
    lgt = nc.dram_tensor("lgt", [B_SH, I], F32, kind="ExternalInput").ap()
    cti = nc.dram_tensor("cti", [C, B_SH], F32, kind="ExternalInput").ap()
    cmT = nc.dram_tensor("cmT", [C, CMS], F32, kind="ExternalInput").ap()
    biasb = nc.dram_tensor("biasb", [P, CMS], F32, kind="ExternalInput").ap()
    tgtT = nc.dram_tensor("tgtT", [P, NB], F32, kind="ExternalInput").ap()
    # wT pre-laid-out by host: wT[p, t*R + r] = weights[r, t*128 + p]
    wT = nc.dram_tensor("wT", [P, NT * R], F32, kind="ExternalInput").ap()
    iota16 = nc.dram_tensor("iota16", [P, R], F32, kind="ExternalInput").ap()
    pow2 = nc.dram_tensor("pow2", [P, CMS], F32, kind="ExternalInput").ap()
    id16 = nc.dram_tensor("id16", [R, R], F32, kind="ExternalInput").ap()

    # ---- per-core DRAM outputs ----
    outT = nc.dram_tensor("outT", [P, NB], F32, kind="ExternalOutput").ap()
    delta = nc.dram_tensor("delta", [R, I], F32, kind="ExternalOutput").ap()

    with tile.TileContext(nc) as tc:
        with (
            tc.tile_pool(name="consts", bufs=1) as consts,
            tc.tile_pool(name="lgp", bufs=3) as lgp,
            tc.tile_pool(name="lgtp", bufs=NB) as lgtp,
            tc.tile_pool(name="work", bufs=1) as work,
            tc.tile_pool(name="small", bufs=2) as small,
        ):
            # ---- constants / small inputs ----
            cti_sb = consts.tile([C, B_SH], F32, tag="cti")
            nc.sync.dma_start(out=cti_sb[:], in_=cti[:])
            cmT_sb = consts.tile([C, CMS], F32, tag="cmT")
            nc.sync.dma_start(out=cmT_sb[:], in_=cmT[:])
            biasb_sb = consts.tile([P, CMS], F32, tag="biasb")
            nc.sync.dma_start(out=biasb_sb[:], in_=biasb[:])
            tgt_sb = consts.tile([P, NB], F32, tag="tgt")
            nc.sync.dma_start(out=tgt_sb[:], in_=tgtT[:])
            wT_sb = consts.tile([P, NT * R], F32, tag="wT")
            nc.sync.dma_start(out=wT_sb[:], in_=wT[:])
            iota_sb = consts.tile([P, R], F32, tag="iota")
            nc.sync.dma_start(out=iota_sb[:], in_=iota16[:])
            pow2_sb = consts.tile([P, CMS], F32, tag="pow2")
            nc.sync.dma_start(out=pow2_sb[:], in_=pow2[:])
            id16_sb = consts.tile([R, R], F32, tag="id16")
            nc.sync.dma_start(out=id16_sb[:], in_=id16[:])

            # ---- resident logitsT tiles (streamed in early, used by mm2) ----
            lgt_tiles = []
            for n in range(NB):
                t_ = lgtp.tile([P, I], F32, tag="lgt", name=f"lgt{n}")
                nc.sync.dma_start(out=t_[:], in_=lgt[n * P : (n + 1) * P, :])
                lgt_tiles.append(t_)

            # ---- persistent work tiles ----
            idxT = work.tile([P, NB], F32, tag="idxT")
            onehotT = work.tile([P, NB * R], F32, tag="onehotT")
            alloutT = work.tile([P, NB * R], F32, tag="alloutT")
            coefT = work.tile([P, NB * R], F32, tag="coefT")
            outv = work.tile([P, NB], F32, tag="outv")
            sig = work.tile([P, NB], F32, tag="sig")
            gv = work.tile([P, NB], F32, tag="gv")
            allout_sb = work.tile([R, B_SH], F32, tag="allout_sb")
            delta_sb = work.tile([R, I], F32, tag="delta_sb")

            with tc.tile_pool(name="ps1", bufs=2, space="PSUM") as ps1, \
                 tc.tile_pool(name="psA", bufs=1, space="PSUM") as psA:
                # ---- context hashing (batch-major) ----
                for n in range(NB):
                    ps_dist = ps1.tile([P, CMS], F32, tag="dist")
                    nc.tensor.matmul(
                        ps_dist[:],
                        lhsT=cti_sb[:, n * P : (n + 1) * P],
                        rhs=cmT_sb[:],
                        start=True,
                        stop=True,
                    )
                    bits = small.tile([P, CMS], F32, tag="bits")
                    nc.vector.tensor_tensor(
                        out=bits[:], in0=ps_dist[:], in1=biasb_sb[:],
                        op=mybir.AluOpType.is_gt,
                    )
                    bp = small.tile([P, CMS], F32, tag="bp")
                    nc.vector.tensor_mul(out=bp[:], in0=bits[:], in1=pow2_sb[:])
                    nc.vector.tensor_reduce(
                        out=idxT[:, n : n + 1], in_=bp[:],
                        axis=mybir.AxisListType.X, op=mybir.AluOpType.add,
                    )
                    nc.vector.tensor_scalar(
                        out=onehotT[:, n * R : (n + 1) * R],
                        in0=iota_sb[:],
                        scalar1=idxT[:, n : n + 1],
                        scalar2=None,
                        op0=mybir.AluOpType.is_equal,
                    )

                # ---- mm1: allout (16, B_SH) accumulated over 32 input chunks ----
                ps_allout = psA.tile([R, B_SH], F32, tag="allout")
                for t in range(NT):
                    lg_t = lgp.tile([P, B_SH], F32, tag="lg")
                    nc.sync.dma_start(out=lg_t[:], in_=lg[t * P : (t + 1) * P, :])
                    for s in range(B_SH // 512):
                        nc.tensor.matmul(
                            ps_allout[:, s * 512 : (s + 1) * 512],
                            lhsT=wT_sb[:, t * R : (t + 1) * R],
                            rhs=lg_t[:, s * 512 : (s + 1) * 512],
                            start=(t == 0),
                            stop=(t == NT - 1),
                        )
                nc.vector.tensor_copy(out=allout_sb[:], in_=ps_allout[:])

                # ---- transpose allout to batch-major (PE transpose via id16) ----
                for n in range(NB):
                    ps_aT = ps1.tile([P, R], F32, tag="aT")
                    nc.tensor.transpose(
                        out=ps_aT[:],
                        in_=allout_sb[:, n * P : (n + 1) * P],
                        identity=id16_sb[:],
                    )
                    nc.scalar.copy(
                        out=alloutT[:, n * R : (n + 1) * R], in_=ps_aT[:]
                    )

            # ---- out = sum_r onehot*allout ; clip ; sigmoid ; g ----
            prod = work.tile([P, NB * R], F32, tag="prod")
            nc.vector.tensor_mul(out=prod[:], in0=onehotT[:], in1=alloutT[:])
            nc.vector.tensor_reduce(
                out=outv[:].rearrange("p (n o) -> p n o", o=1),
                in_=prod[:].rearrange("p (n r) -> p n r", r=R),
                axis=mybir.AxisListType.X,
                op=mybir.AluOpType.add,
            )
            nc.vector.tensor_scalar(
                out=outv[:], in0=outv[:],
                scalar1=CLIP_LO, scalar2=CLIP_HI,
                op0=mybir.AluOpType.max, op1=mybir.AluOpType.min,
            )
            nc.sync.dma_start(out=outT[:], in_=outv[:])
            nc.scalar.activation(
                out=sig[:], in_=outv[:], func=mybir.ActivationFunctionType.Sigmoid,
            )
            nc.vector.tensor_sub(out=gv[:], in0=sig[:], in1=tgt_sb[:])
            nc.vector.tensor_scalar(
                out=gv[:], in0=gv[:], scalar1=LR, scalar2=None,
                op0=mybir.AluOpType.mult,
            )
            # coefT = onehotT * g (per-partition scalar per batch chunk)
            for n in range(NB):
                nc.vector.tensor_scalar(
                    out=coefT[:, n * R : (n + 1) * R],
                    in0=onehotT[:, n * R : (n + 1) * R],
                    scalar1=gv[:, n : n + 1],
                    scalar2=None,
                    op0=mybir.AluOpType.mult,
                )

            # ---- mm2: delta (16, I) += coefT.T @ logitsT over batch chunks ----
            with tc.tile_pool(name="psD", bufs=8, space="PSUM") as psD:
                ps_d = [psD.tile([R, 512], F32, tag="d", name=f"psd{s_}")
                        for s_ in range(8)]
                for n in range(NB):
                    for s in range(8):
                        nc.tensor.matmul(
                            ps_d[s][:],
                            lhsT=coefT[:, n * R : (n + 1) * R],
                            rhs=lgt_tiles[n][:, s * 512 : (s + 1) * 512],
                            start=(n == 0),
                            stop=(n == NB - 1),
                        )
                for s in range(8):
                    nc.vector.tensor_copy(
                        out=delta_sb[:, s * 512 : (s + 1) * 512], in_=ps_d[s][:]
                    )
            nc.sync.dma_start(out=delta[:], in_=delta_sb[:])

    return nc


_CACHE = {}


def _get_nc():
    if "nc" not in _CACHE:
        nc = bacc.Bacc("TRN2", target_bir_lowering=False, debug=False,
                       num_devices=N_CORES)
        nc.name = "nn_neuron_kernel"
        build(nc)
        nc.compile()
        _CACHE["nc"] = nc
    return _CACHE["nc"]


def make_in_maps(logits, context_inputs, targets, context_maps, context_bias,
                 weights):
    logits = np.asarray(logits, np.float32)
    context_inputs = np.asarray(context_inputs, np.float32)
    targets = np.asarray(targets, np.float32)
    context_maps = np.asarray(context_maps, np.float32)
    context_bias = np.asarray(context_bias, np.float32)
    weights = np.asarray(weights, np.float32)

    lgt_full = np.ascontiguousarray(logits.T)
    cmT = np.ascontiguousarray(context_maps.T)
    biasb = np.ascontiguousarray(
        np.broadcast_to(context_bias.reshape(1, CMS), (P, CMS))
    )
    # (P, NT*R) with wT[p, t*R + r] = weights[r, t*128 + p]
    wT = np.ascontiguousarray(
        weights.T.reshape(NT, P, R).transpose(1, 0, 2).reshape(P, NT * R)
    )
    iota16 = np.ascontiguousarray(
        np.broadcast_to(np.arange(R, dtype=np.float32), (P, R))
    )
    pow2 = np.ascontiguousarray(
        np.broadcast_to((2.0 ** np.arange(CMS)).astype(np.float32), (P, CMS))
    )
    id16 = np.eye(R, dtype=np.float32)

    in_maps = []
    for c in range(N_CORES):
        sl = slice(c * B_SH, (c + 1) * B_SH)
        in_maps.append({
            "lg": np.ascontiguousarray(logits[:, sl]),
            "lgt": np.ascontiguousarray(lgt_full[sl, :]),
            "cti": np.ascontiguousarray(context_inputs[:, sl]),
            "cmT": cmT,
            "biasb": biasb,
            "tgtT": np.ascontiguousarray(targets[sl].reshape(NB, P).T),
            "wT": wT,
            "iota16": iota16,
            "pow2": pow2,
            "id16": id16,
        })
    return in_maps


def assemble(results, weights):
    out_full = np.concatenate(
        [np.ascontiguousarray(r["outT"].T).reshape(-1) for r in results]
    )
    dsum = np.zeros((R, I), np.float32)
    for r in results:
        dsum += r["delta"]
    new_w = np.clip(np.asarray(weights, np.float32) - dsum, -WCLIP, WCLIP)
    return out_full, new_w


def kernel(logits, context_inputs, targets, context_maps, context_bias,
           weights):
    nc = _get_nc()
    in_maps = make_in_maps(logits, context_inputs, targets, context_maps,
                           context_bias, weights)
    res = run_bass_kernel_spmd(nc, in_maps, core_ids=list(range(N_CORES)))
    return assemble(res.results, weights)
